# revision 24
# baseline (speedup 1.0000x reference)
"""ExpireSpanGPT Bass/Tile kernel for 8 Trainium2 NeuronCores.

Sharding: 2 groups x 4 cores; group g owns batch b=g. Within a group
(TP-4): each core owns 2 heads. Per layer: local-head attention ->
Wo-projection partials (+0.25*x residual) -> ReduceScatter(add) hands
each core its 256-token slice -> MLP on the slice -> AllGather restores
full x^T. Head phase: each core computes logits for its batch over a
8000-wide vocab slice (padded to 8192).

Activations live transposed in SBUF: x^T [128(d-part), kt, tok].
TXL relative-position skew via strided DRAM round trip: Qp rows written
at stride 1152, read back at stride 1151; sentinel columns (-60000)
double as the causal mask.
"""

import numpy as np

# --- NTFF profiling shim: register antenv.axon_hooks if absent -------------
import contextlib as _ctxlib
import ctypes as _ctypes
import os as _os
import sys as _sys
import types as _types


def _make_ntff_hook():
    so = _os.environ.get("AXON_PJRT_SO", "/opt/axon/libaxon_pjrt.so")
    if not _os.path.exists(so):
        return None
    try:
        lib = _ctypes.CDLL(so)
    except OSError:
        return None
    if not hasattr(lib, "axon_start_nrt_profile"):
        return None
    lib.axon_start_nrt_profile.argtypes = [
        _ctypes.POINTER(_ctypes.c_int64), _ctypes.c_size_t]
    lib.axon_start_nrt_profile.restype = _ctypes.c_int64
    lib.axon_stop_nrt_profile.argtypes = [_ctypes.c_char_p]
    lib.axon_stop_nrt_profile.restype = _ctypes.c_int64

    @_ctxlib.contextmanager
    def _hook(output_dir, device_ids):
        import jax
        jax.devices()
        if device_ids:
            ids = (_ctypes.c_int64 * len(device_ids))(*device_ids)
            rc = lib.axon_start_nrt_profile(ids, len(device_ids))
        else:
            rc = lib.axon_start_nrt_profile(None, 0)
        if rc != 0:
            raise RuntimeError(f"axon_start_nrt_profile rc={rc}")
        try:
            yield
        finally:
            n = lib.axon_stop_nrt_profile(output_dir.encode())
            if n < 0:
                raise RuntimeError(f"axon_stop_nrt_profile rc={n}")

    return _hook


try:
    from antenv.axon_hooks import get_axon_ntff_profile_hook  # noqa: F401
except ImportError:
    _m = _types.ModuleType("antenv.axon_hooks")
    _NTFF_HOOK = [None, False]

    def set_axon_ntff_profile_hook(hook):
        _NTFF_HOOK[0] = hook
        _NTFF_HOOK[1] = True

    def get_axon_ntff_profile_hook():
        if not _NTFF_HOOK[1]:
            _NTFF_HOOK[0] = _make_ntff_hook()
            _NTFF_HOOK[1] = True
        return _NTFF_HOOK[0]

    _m.set_axon_ntff_profile_hook = set_axon_ntff_profile_hook
    _m.get_axon_ntff_profile_hook = get_axon_ntff_profile_hook
    _sys.modules["antenv.axon_hooks"] = _m
    try:
        import antenv as _antenv
        _antenv.axon_hooks = _m
    except ImportError:
        pass
# ---------------------------------------------------------------------------

import concourse.bass as bass
import concourse.tile as tile
from concourse import bacc, mybir
from concourse.bass_utils import run_bass_kernel_spmd

FP = mybir.dt.float32
FPR = mybir.dt.float32r
F16 = mybir.dt.float16
I16 = mybir.dt.int16

DEPTH, D, H, DH = 4, 512, 8, 64
N, M, V = 1024, 1024, 32000
KT = D // 128
NC_ = 8
GROUPS = [[0, 1, 2, 3], [4, 5, 6, 7]]
VPAD = 8192
TOKSL = N // 4
QSTRIDE = 1152
SENT = -60000.0
AF = mybir.ActivationFunctionType
OP = mybir.AluOpType

_CACHE = {}


def _sin_emb(n, dim):
    inv = 1.0 / (10000.0 ** (np.arange(0, dim, 2, dtype=np.float64) / dim))
    t = np.arange(n - 1, -1, -1, dtype=np.float64)
    s = t[:, None] * inv[None, :]
    return np.concatenate([np.sin(s), np.cos(s)], axis=-1).astype(np.float32)


def build_program(depth=DEPTH, debug=False):
    nc = bacc.Bacc("TRN2", target_bir_lowering=False, debug=False,
                   num_devices=NC_)

    def din(name, shape, dt=FP):
        return nc.dram_tensor(name, list(shape), dt, kind="ExternalInput")

    def dout(name, shape, dt=FP):
        return nc.dram_tensor(name, list(shape), dt, kind="ExternalOutput")

    tok_emb = din("tok_emb", [V, D])
    tok_idx = din("tok_idx", [128, 64], I16)
    memT = din("memT", [DEPTH, 128, KT, M])
    tpre = din("tpre", [128, DEPTH * 8])
    wq = din("wq", [DEPTH, 128, KT, 128], F16)
    wk = din("wk", [DEPTH, 128, KT, 128], F16)
    wv = din("wv", [DEPTH, 128, KT, 128], F16)
    wpos = din("wpos", [DEPTH, 128, KT, 64], F16)
    wexp = din("wexp", [128, DEPTH, KT])
    wo = din("wo", [DEPTH, 128, KT, 128], F16)   # local-head rows
    w1 = din("w1", [DEPTH, 128, KT, 2048], F16)
    w2 = din("w2", [DEPTH, 128, 16, 512], F16)
    whead = din("whead", [128, KT, VPAD], F16)
    bq = din("bq", [128, DEPTH])
    bk = din("bk", [128, DEPTH])
    bv = din("bv", [128, DEPTH])
    bpos = din("bpos", [64, DEPTH])
    bexp = din("bexp", [128, DEPTH])
    boq = din("boq", [128, DEPTH * KT])      # bo/4: [p, l*4+ct]
    b1 = din("b1", [128, DEPTH * 16])
    b2 = din("b2", [128, DEPTH * KT])
    lng = din("lng", [128, KT])
    lnb = din("lnb", [128, KT])

    logits_part = dout("logits_part", [N, VPAD])
    aux_part = dout("aux_part", [1, 1])
    dbg = {}
    if debug:
        for l in range(depth + 1):
            dbg[l] = dout(f"dbg_x{l}", [128, KT, N])
        dbg["em"] = dout("dbg_em", [128, DEPTH * 8])
        dbg["ao"] = dout("dbg_ao", [128, N])
        dbg["yb"] = dout("dbg_yb", [128, KT, N])
        dbg["mi"] = dout("dbg_mi", [128, KT, TOKSL])

    peT_np = np.zeros((128, KT, N), np.float32)
    pe = _sin_emb(N, D)
    for kt in range(KT):
        peT_np[:, kt, :] = pe[:, kt * 128:(kt + 1) * 128].T
    peT_d = nc.inline_tensor(peT_np.astype(np.float16), "peT_c")
    ident_d = nc.inline_tensor(np.eye(128, dtype=np.float32), "ident_c")
    ones1_d = nc.inline_tensor(np.ones((128, 1), np.float32), "ones1_c")
    onesr_d = nc.inline_tensor(np.ones((1, 128), np.float32), "onesr_c")

    from contextlib import ExitStack
    with tile.TileContext(nc) as tc, ExitStack() as ctx:
        cpool = ctx.enter_context(tc.tile_pool(name="const", bufs=1))
        spool = ctx.enter_context(tc.tile_pool(name="state", bufs=1))
        lpool = ctx.enter_context(tc.tile_pool(name="layer", bufs=1))
        mpool = ctx.enter_context(tc.tile_pool(name="mem", bufs=1))
        apool = ctx.enter_context(tc.tile_pool(name="attn", bufs=3))
        epool = ctx.enter_context(tc.tile_pool(name="attnE", bufs=2))
        wpool = ctx.enter_context(tc.tile_pool(name="wstream", bufs=3))
        psA = ctx.enter_context(tc.tile_pool(name="psA", bufs=3, space="PSUM"))
        psB = ctx.enter_context(tc.tile_pool(name="psB", bufs=2, space="PSUM"))
        psC = ctx.enter_context(tc.tile_pool(name="psC", bufs=1, space="PSUM"))
        psD = ctx.enter_context(tc.tile_pool(name="psD", bufs=2, space="PSUM"))
        dram = ctx.enter_context(tc.tile_pool(name="dram", bufs=1,
                                              space="DRAM"))
        qdram = ctx.enter_context(tc.tile_pool(name="qdram", bufs=8,
                                               space="DRAM"))
        hpool = ctx.enter_context(tc.tile_pool(name="whd", bufs=2))

        def mm512():
            return psA.tile([128, 512], FP, tag="mm512", name="mm512")

        def mmsmall():
            return psB.tile([128, 128], FP, tag="small", name="small")

        peT = cpool.tile([128, KT, N], F16, tag="peT")
        nc.sync.dma_start(peT[:], peT_d[:])
        ident = cpool.tile([128, 128], FP, tag="ident")
        nc.sync.dma_start(ident[:], ident_d[:])
        ones1 = cpool.tile([128, 1], FP, tag="ones1")
        nc.sync.dma_start(ones1[:], ones1_d[:])
        onesr = cpool.tile([1, 128], FP, tag="onesr")
        nc.sync.dma_start(onesr[:], onesr_d[:])
        sent = cpool.tile([128, 128], F16, tag="sent")
        nc.vector.memset(sent[:], SENT)

        def cload(name, src, shape, dt=FP):
            t = cpool.tile(list(shape), dt, tag=name, name=name)
            nc.sync.dma_start(t[:], src[:])
            return t

        tpre_s = cload("tpre", tpre, [128, DEPTH * 8])
        bq_s = cload("bq", bq, [128, DEPTH])
        bk_s = cload("bk", bk, [128, DEPTH])
        bv_s = cload("bv", bv, [128, DEPTH])
        bpos_s = cload("bpos", bpos, [64, DEPTH])
        bexp_s = cload("bexp", bexp, [128, DEPTH])
        boq_s = cload("boq", boq, [128, DEPTH * KT])
        b1_s = cload("b1", b1, [128, DEPTH * 16])
        b2_s = cload("b2", b2, [128, DEPTH * KT])
        lng_s = cload("lng", lng, [128, KT])
        lnb_s = cload("lnb", lnb, [128, KT])
        wexp_s = cload("wexp", wexp, [128, DEPTH, KT])
        idx_s = cpool.tile([128, 64], I16, tag="idx")
        nc.sync.dma_start(idx_s[:], tok_idx[:])

        qpfs = []
        for qi in range(8):
            qf = qdram.tile([128 * QSTRIDE + 1024], F16, tag="qpf")
            qv = qf[:128 * QSTRIDE].rearrange("(r s) -> r s", s=QSTRIDE)
            nc.sync.dma_start(qv[:, 1024:1152], sent[:])
            qpfs.append(qf)

        # ---- embedding gather + transpose into xT ----
        xT = spool.tile([128, KT, N], FP, tag="xT")
        x0 = spool.tile([128, 8, D], FP, tag="ybuf")
        nc.gpsimd.dma_gather(
            out_ap=x0[:], in_ap=tok_emb[:], idxs_ap=idx_s[:],
            num_idxs=N, num_idxs_reg=N, elem_size=D)
        for g in range(8):
            for kt in range(KT):
                ps = mmsmall()
                nc.tensor.transpose(ps[:], x0[:, g, kt * 128:(kt + 1) * 128],
                                    ident[:])
                nc.vector.tensor_copy(
                    out=xT[:, kt, g * 128:(g + 1) * 128], in_=ps[:])
        if debug:
            nc.sync.dma_start(dbg[0][:], xT[:])

        aux_acc = spool.tile([128, 1], FP, tag="aux")
        nc.vector.memset(aux_acc[:], 0.0)
        ybuf = spool.tile([128, KT, N], FP, tag="ybuf")

        for l in range(depth):
            memT_l = mpool.tile([128, KT, M], FP, tag="memT")
            nc.sync.dma_start(memT_l[:], memT[l])
            wq_l = lpool.tile([128, KT, 128], F16, tag="wq")
            nc.sync.dma_start(wq_l[:], wq[l])
            wk_l = lpool.tile([128, KT, 128], F16, tag="wk")
            nc.sync.dma_start(wk_l[:], wk[l])
            wv_l = lpool.tile([128, KT, 128], F16, tag="wv")
            nc.sync.dma_start(wv_l[:], wv[l])
            wpos_l = lpool.tile([128, KT, 64], F16, tag="wpos")
            nc.sync.dma_start(wpos_l[:], wpos[l])
            wo_l = lpool.tile([128, KT, 128], F16, tag="wo")
            nc.sync.dma_start(wo_l[:], wo[l])

            # ---- expire-span gating ----
            sig = lpool.tile([128, 8], FP, tag="sig")
            for jt in range(8):
                pse = psB.tile([128, 1], FP, tag="small", name="exps")
                for kt in range(KT):
                    nc.tensor.matmul(
                        pse[:], lhsT=memT_l[:, kt, jt * 128:(jt + 1) * 128],
                        rhs=wexp_s[:, l, kt:kt + 1],
                        start=(kt == 0), stop=(kt == KT - 1))
                nc.scalar.activation(sig[:, jt:jt + 1], pse[:], AF.Sigmoid,
                                     bias=bexp_s[:, l:l + 1])
            em = lpool.tile([128, 8], FP, tag="em")
            nc.vector.scalar_tensor_tensor(
                em[:], sig[:], 8.0, tpre_s[:, l * 8:(l + 1) * 8],
                OP.mult, OP.add)
            nc.vector.tensor_scalar(em[:], em[:], 1.0, 0.0, OP.min, OP.max)
            if debug:
                nc.sync.dma_start(dbg["em"][:, l * 8:(l + 1) * 8], em[:])
            ind = lpool.tile([128, 8], FP, tag="ind")
            nc.vector.tensor_scalar(ind[:], em[:], 0.0, None, OP.is_gt)
            i2 = lpool.tile([128, 8], FP, tag="ind2")
            nc.vector.tensor_scalar(i2[:], em[:], 1.0, None, OP.is_lt)
            nc.vector.tensor_tensor(ind[:], ind[:], i2[:], OP.mult)
            nc.vector.tensor_tensor(ind[:], ind[:], sig[:], OP.mult)
            red = lpool.tile([128, 1], FP, tag="red")
            nc.vector.tensor_reduce(red[:], ind[:], mybir.AxisListType.X,
                                    OP.add)
            nc.vector.scalar_tensor_tensor(
                aux_acc[:], red[:], 1.0 / 128.0, aux_acc[:], OP.mult, OP.add)

            x16 = lpool.tile([128, KT, N], F16, tag="x16")
            nc.vector.tensor_copy(out=x16[:], in_=xT[:])
            mem16 = lpool.tile([128, KT, M], F16, tag="mem16")
            nc.vector.tensor_copy(out=mem16[:], in_=memT_l[:])

            # ---- kT, vT, qT, posT ----
            kT = lpool.tile([128, 4, 512], F16, tag="kT")
            v_nat = lpool.tile([128, 16, 128], F16, tag="vnat")
            for jc in range(4):
                srct = mem16 if jc < 2 else x16
                off = (jc % 2) * 512
                ps = mm512()
                for kt in range(KT):
                    nc.tensor.matmul(
                        ps[:], lhsT=wk_l[:, kt, :],
                        rhs=srct[:, kt, off:off + 512],
                        start=(kt == 0), stop=(kt == KT - 1))
                nc.vector.tensor_scalar(kT[:, jc, :], ps[:],
                                        bk_s[:, l:l + 1], None, OP.add)
                psv = mm512()
                for kt in range(KT):
                    nc.tensor.matmul(
                        psv[:], lhsT=wv_l[:, kt, :],
                        rhs=srct[:, kt, off:off + 512],
                        start=(kt == 0), stop=(kt == KT - 1))
                vc = lpool.tile([128, 512], FP, tag="vTc")
                nc.vector.tensor_scalar(vc[:], psv[:],
                                        bv_s[:, l:l + 1], None, OP.add)
                for sub in range(4):
                    jt = jc * 4 + sub
                    pst = mmsmall()
                    nc.tensor.transpose(
                        pst[:], vc[:, sub * 128:(sub + 1) * 128], ident[:])
                    if jt < 8:
                        nc.vector.tensor_scalar(v_nat[:, jt, :], pst[:],
                                                em[:, jt:jt + 1], None,
                                                OP.mult)
                    else:
                        nc.vector.tensor_copy(out=v_nat[:, jt, :], in_=pst[:])
            qT = lpool.tile([128, 2, 512], F16, tag="qT")
            for ic in range(2):
                ps = mm512()
                for kt in range(KT):
                    nc.tensor.matmul(
                        ps[:], lhsT=wq_l[:, kt, :],
                        rhs=x16[:, kt, ic * 512:(ic + 1) * 512],
                        start=(kt == 0), stop=(kt == KT - 1))
                nc.vector.tensor_scalar(qT[:, ic, :], ps[:], bq_s[:, l:l + 1],
                                        None, OP.add)
            posT = lpool.tile([128, 2, 512], F16, tag="posT")
            for ic in range(2):
                ps = mm512()
                for kt in range(KT):
                    nc.tensor.matmul(
                        ps[:64, :], lhsT=wpos_l[:, kt, :],
                        rhs=peT[:, kt, ic * 512:(ic + 1) * 512],
                        start=(kt == 0), stop=(kt == KT - 1))
                nc.vector.tensor_scalar(posT[:64, ic, :], ps[:64, :],
                                        bpos_s[:, l:l + 1], None, OP.add)
                nc.sync.dma_start(posT[64:, ic, :], posT[:64, ic, :])

            # ---- attention (2 local heads) ----
            aoT_loc = lpool.tile([128, N], F16, tag="aoT_loc")
            for hh in range(2):
                qr = 64 * hh
                for it in range(8):
                    i0 = it * 128
                    qsl = qT[qr:qr + 64, it // 4,
                             (it % 4) * 128:(it % 4 + 1) * 128]
                    width = 1024 + 128 * (it + 1)
                    jtiles = width // 128
                    njc = (width + 511) // 512
                    lo_cc = (1024 - 128 * (it + 1)) // 512
                    qpf = qpfs[(hh * 8 + it) % 8]
                    qv = qpf[:128 * QSTRIDE].rearrange("(r s) -> r s", s=QSTRIDE)
                    qp16 = apool.tile([128, 1024], F16, tag="qp16")
                    for cc in range(lo_cc, 2):
                        ps = mm512()
                        nc.tensor.matmul(ps[:], lhsT=qsl,
                                         rhs=posT[qr:qr + 64, cc, :],
                                         start=True, stop=True)
                        nc.vector.tensor_copy(
                            out=qp16[:, cc * 512:(cc + 1) * 512], in_=ps[:])
                        nc.scalar.dma_start(
                            qv[:, cc * 512:(cc + 1) * 512],
                            qp16[:, cc * 512:(cc + 1) * 512])
                    pd = apool.tile([128, 1024], F16, tag="pd")
                    pw = 128 * (it + 1)
                    base = 1023 - i0
                    shear = qpf[base: base + 128 * (QSTRIDE - 1)
                                ].rearrange("(r s) -> r s", s=QSTRIDE - 1)
                    nc.scalar.dma_start(pd[:, :pw], shear[:, :pw])

                    E = epool.tile([128, 2048], F16, tag="E")
                    dens = apool.tile([128, 4], FP, tag="dens")
                    for jc in range(njc):
                        w = min(512, width - jc * 512)
                        ps = mm512()
                        nc.tensor.matmul(
                            ps[:, :w], lhsT=qsl,
                            rhs=kT[qr:qr + 64, jc, :w],
                            start=True, stop=True)
                        jj0 = max(0, jc * 512 - 1024)
                        jj1 = min(pw, (jc + 1) * 512 - 1024)
                        if jj1 > jj0:
                            c0 = 1024 + jj0 - jc * 512
                            nc.vector.tensor_tensor(
                                ps[:, c0:c0 + (jj1 - jj0)],
                                ps[:, c0:c0 + (jj1 - jj0)],
                                pd[:, jj0:jj1], OP.add)
                        nc.scalar.activation(
                            E[:, jc * 512:jc * 512 + w], ps[:, :w], AF.Exp,
                            accum_out=dens[:, jc:jc + 1])
                    ET = epool.tile([128, 16, 128], F16, tag="ET")
                    nc.scalar.dma_start_transpose(ET[:, :jtiles, :],
                                                E[:, :width])
                    den1 = apool.tile([128, 1], FP, tag="den1")
                    nc.vector.tensor_reduce(den1[:], dens[:, :njc],
                                            mybir.AxisListType.X, OP.add)
                    rinv = apool.tile([128, 1], FP, tag="rinv")
                    nc.vector.reciprocal(rinv[:], den1[:])
                    po = psD.tile([128, 64], FP, tag="po", name="po")
                    for jt in range(jtiles):
                        nc.tensor.matmul(
                            po[:, :64], lhsT=ET[:, jt, :],
                            rhs=v_nat[:, jt, qr:qr + 64],
                            start=(jt == 0), stop=(jt == jtiles - 1))
                    att = apool.tile([128, 64], FP, tag="att")
                    nc.vector.tensor_scalar(att[:], po[:, :64], rinv[:],
                                            None, OP.mult)
                    pt = mmsmall()
                    nc.tensor.transpose(pt[:64, :], att[:], ident[:])
                    nc.vector.tensor_copy(
                        out=aoT_loc[qr:qr + 64, i0:i0 + 128], in_=pt[:64, :])
            if debug and l == 0:
                nc.sync.dma_start(dbg["ao"][:], aoT_loc[:])

            # ---- Wo projection partials + 0.25*x + bo/4 ----
            for ct in range(KT):
                for ic in range(2):
                    ps = mm512()
                    nc.tensor.matmul(
                        ps[:], lhsT=wo_l[:, ct, :],
                        rhs=aoT_loc[:, ic * 512:(ic + 1) * 512],
                        start=True, stop=True)
                    sl = slice(ic * 512, (ic + 1) * 512)
                    nc.vector.scalar_tensor_tensor(
                        ybuf[:, ct, sl], xT[:, ct, sl], 0.25, ps[:],
                        OP.mult, OP.add)
                nc.vector.tensor_scalar(
                    ybuf[:, ct, :], ybuf[:, ct, :],
                    boq_s[:, l * KT + ct:l * KT + ct + 1], None, OP.add)

            if debug and l == 0:
                nc.sync.dma_start(dbg["yb"][:], ybuf[:])
            # ---- ReduceScatter -> my 256-token slice ----
            rs_in = dram.tile([4, 128, KT, TOKSL], FP, tag="rsi")
            rs_out = dram.tile([128, KT, TOKSL], FP, tag="rso")
            for sr in range(4):
                nc.gpsimd.dma_start(
                    rs_in[sr], ybuf[:, :, sr * TOKSL:(sr + 1) * TOKSL])
            nc.gpsimd.collective_compute(
                "ReduceScatter", OP.add, replica_groups=GROUPS,
                ins=[rs_in.opt()], outs=[rs_out.opt()])
            mlp_in = lpool.tile([128, KT, TOKSL], FP, tag="mlpin")
            nc.sync.dma_start(mlp_in[:], rs_out[:])
            if debug and l == 0:
                nc.sync.dma_start(dbg["mi"][:], mlp_in[:])

            mlp16 = lpool.tile([128, KT, TOKSL], F16, tag="mlp16")
            nc.vector.tensor_copy(out=mlp16[:], in_=mlp_in[:])

            # ---- MLP on slice ----
            h1full = lpool.tile([128, 16, TOKSL], F16, tag="h1")
            for hc in range(16):
                w1t = wpool.tile([128, KT, 128], F16, tag="w1t")
                nc.sync.dma_start(w1t[:],
                                  w1[l, :, :, hc * 128:(hc + 1) * 128])
                ps = mm512()
                for kt in range(KT):
                    nc.tensor.matmul(
                        ps[:, :TOKSL], lhsT=w1t[:, kt, :],
                        rhs=mlp16[:, kt, :],
                        start=(kt == 0), stop=(kt == KT - 1))
                nc.scalar.activation(
                    h1full[:, hc, :], ps[:, :TOKSL], AF.Gelu,
                    bias=b1_s[:, l * 16 + hc:l * 16 + hc + 1])
            xout = lpool.tile([128, KT, TOKSL], FP, tag="xout")
            for ct in range(KT):
                w2t = hpool.tile([128, 16, 128], F16, tag="w2t")
                nc.sync.dma_start(w2t[:], w2[l, :, :, ct * 128:(ct + 1) * 128])
                zp = psC.tile([128, 256], FP, tag="acc256", name="zp")
                for hc in range(16):
                    nc.tensor.matmul(
                        zp[:], lhsT=w2t[:, hc, :],
                        rhs=h1full[:, hc, :],
                        start=(hc == 0), stop=(hc == 15))
                nc.vector.scalar_tensor_tensor(
                    xout[:, ct, :], zp[:],
                    b2_s[:, l * KT + ct:l * KT + ct + 1],
                    mlp_in[:, ct, :], OP.add, OP.add)

            # ---- AllGather token slices -> full xT ----
            ag_in = dram.tile([128, KT, TOKSL], FP, tag="agi")
            ag_out = dram.tile([4, 128, KT, TOKSL], FP, tag="ago")
            nc.gpsimd.dma_start(ag_in[:], xout[:])
            nc.gpsimd.collective_compute(
                "AllGather", OP.bypass, replica_groups=GROUPS,
                ins=[ag_in.opt()], outs=[ag_out.opt()])
            for sr in range(4):
                nc.sync.dma_start(
                    xT[:, :, sr * TOKSL:(sr + 1) * TOKSL], ag_out[sr])
            if debug:
                nc.sync.dma_start(dbg[l + 1][:], xT[:])

        # ---- final layernorm (feature dim = partitions) ----
        xsq = spool.tile([128, KT, N], FP, tag="ybuf2")
        nc.scalar.activation(xsq[:], xT[:], AF.Square)
        mean1 = spool.tile([1, N], FP, tag="mean1")
        scr = spool.tile([1, N], FP, tag="scr")
        rstd1 = spool.tile([1, N], FP, tag="rstd1")
        for ic in range(2):
            sl = slice(ic * 512, (ic + 1) * 512)
            ps = mm512()
            for kt in range(KT):
                nc.tensor.matmul(
                    ps[:1, :], lhsT=ones1[:],
                    rhs=xT[:, kt, sl],
                    start=(kt == 0), stop=(kt == KT - 1))
            nc.vector.tensor_scalar(mean1[:, sl], ps[:1, :], 1.0 / 512.0,
                                    None, OP.mult)
            ps2 = mm512()
            for kt in range(KT):
                nc.tensor.matmul(
                    ps2[:1, :], lhsT=ones1[:],
                    rhs=xsq[:, kt, sl],
                    start=(kt == 0), stop=(kt == KT - 1))
            nc.vector.tensor_scalar(scr[:, sl], ps2[:1, :], 1.0 / 512.0,
                                    None, OP.mult)
        # var = E[x^2] - mean^2 ; rstd = 1/sqrt(var+eps)
        nc.vector.tensor_tensor(rstd1[:], mean1[:], mean1[:], OP.mult)
        nc.vector.tensor_tensor(scr[:], scr[:], rstd1[:], OP.subtract)
        eps1 = spool.tile([1, 1], FP, tag="eps1")
        nc.vector.memset(eps1[:], 1e-5)
        nc.scalar.activation(rstd1[:], scr[:], AF.Sqrt, bias=eps1[:])
        nc.vector.reciprocal(scr[:], rstd1[:])
        meanb = spool.tile([128, N], FP, tag="meanb")
        rstdb = spool.tile([128, N], FP, tag="rstdb")
        for ic in range(2):
            sl = slice(ic * 512, (ic + 1) * 512)
            ps = mm512()
            nc.tensor.matmul(ps[:], lhsT=onesr[:], rhs=mean1[:, sl],
                             start=True, stop=True)
            nc.vector.tensor_copy(out=meanb[:, sl], in_=ps[:])
            ps2 = mm512()
            nc.tensor.matmul(ps2[:], lhsT=onesr[:], rhs=scr[:, sl],
                             start=True, stop=True)
            nc.vector.tensor_copy(out=rstdb[:, sl], in_=ps2[:])
        xn = spool.tile([128, KT, N], F16, tag="xn16")
        for kt in range(KT):
            nc.vector.tensor_tensor(xn[:, kt, :], xT[:, kt, :], meanb[:],
                                    OP.subtract)
            nc.vector.tensor_tensor(xn[:, kt, :], xn[:, kt, :], rstdb[:],
                                    OP.mult)
            nc.vector.tensor_scalar(xn[:, kt, :], xn[:, kt, :],
                                    lng_s[:, kt:kt + 1], lnb_s[:, kt:kt + 1],
                                    OP.mult, OP.add)

        # ---- head: logits_part[tok, vc] ----
        for vc in range(VPAD // 256):
            wht = hpool.tile([128, KT, 256], F16, tag="wht")
            nc.sync.dma_start(wht[:], whead[:, :, vc * 256:(vc + 1) * 256])
            for tt in range(8):
                ps = mm512()
                for kt in range(KT):
                    nc.tensor.matmul(
                        ps[:, :256], lhsT=xn[:, kt, tt * 128:(tt + 1) * 128
                                             ],
                        rhs=wht[:, kt, :],
                        start=(kt == 0), stop=(kt == KT - 1))
                ob = epool.tile([128, 256], FP, tag="ob")
                nc.vector.tensor_copy(out=ob[:], in_=ps[:, :256])
                nc.scalar.dma_start(
                    logits_part[tt * 128:(tt + 1) * 128,
                                vc * 256:(vc + 1) * 256], ob[:])

        # ---- aux scalar (partition-reduce via ones-matmul) ----
        aux_ps = psB.tile([128, 128], FP, tag="small", name="auxps")
        nc.tensor.matmul(aux_ps[:1, :1], lhsT=ones1[:], rhs=aux_acc[:],
                         start=True, stop=True)
        aux1 = spool.tile([1, 1], FP, tag="aux1")
        nc.vector.tensor_copy(out=aux1[:], in_=aux_ps[:1, :1])
        nc.sync.dma_start(aux_part[:], aux1[:])

    nc.compile()
    return nc


def _wrap_idx(idx):
    """[1024] -> [128, 64] int16 wrapped in 16 partitions, replicated x8."""
    w = idx.astype(np.int16).reshape(64, 16).T          # [16, 64]
    return np.ascontiguousarray(np.tile(w, (8, 1)))     # [128, 64]


def _tile_T(x):
    """[rows, D] -> [128, D//128, rows] transposed-tiled."""
    rows, d = x.shape
    out = np.empty((128, d // 128, rows), np.float32)
    for kt in range(d // 128):
        out[:, kt, :] = x[:, kt * 128:(kt + 1) * 128].T
    return out


def prepare_inputs(core, inputs):
    b, s = core // 4, core % 4
    f0 = 128 * s                      # local head-feature offset (2 heads)
    t0 = TOKSL * s                    # token slice (only used via collective)
    v0 = (V // 4) * s                 # vocab slice

    tokens = np.asarray(inputs["tokens"])[b]
    mems = np.asarray(inputs["mems"], np.float32)
    times = np.asarray(inputs["times"])
    Wq = np.asarray(inputs["Wq"], np.float32)
    bq = np.asarray(inputs["bq"], np.float32)
    Wkv = np.asarray(inputs["Wkv"], np.float32)
    bkv = np.asarray(inputs["bkv"], np.float32)
    Wo = np.asarray(inputs["Wo"], np.float32)
    bo = np.asarray(inputs["bo"], np.float32)
    Wpos = np.asarray(inputs["Wpos"], np.float32)
    bpos = np.asarray(inputs["bpos"], np.float32)
    Wexp = np.asarray(inputs["Wexp"], np.float32)
    bexp = np.asarray(inputs["bexp"], np.float32)
    W1 = np.asarray(inputs["W1"], np.float32)
    b1 = np.asarray(inputs["b1"], np.float32)
    W2 = np.asarray(inputs["W2"], np.float32)
    b2 = np.asarray(inputs["b2"], np.float32)
    ln_g = np.asarray(inputs["ln_g"], np.float32)
    ln_b = np.asarray(inputs["ln_b"], np.float32)
    Whead = np.asarray(inputs["Whead"], np.float32)
    tok_emb = np.asarray(inputs["tok_emb"], np.float32)

    memT = np.stack([_tile_T(mems[l, b]) for l in range(DEPTH)])
    tpre = np.empty((128, DEPTH * 8), np.float32)
    for l in range(DEPTH):
        tl = times[l, b].astype(np.float32)              # [1024]
        tpre[:, l * 8:(l + 1) * 8] = 1.0 - tl.reshape(8, 128).T / RAMP_

    def wslice(W, c0, c1, scale=1.0):
        # W [DEPTH, 512, c] -> [DEPTH, 128, KT, c1-c0]
        out = np.empty((DEPTH, 128, KT, c1 - c0), np.float32)
        for l in range(DEPTH):
            for kt in range(KT):
                out[l, :, kt, :] = W[l, kt * 128:(kt + 1) * 128, c0:c1] * scale
        return np.ascontiguousarray(out)

    wq_h = wslice(Wq, f0, f0 + 128, 0.125).astype(np.float16)
    wk_h = wslice(Wkv, f0, f0 + 128).astype(np.float16)
    wv_h = wslice(Wkv, D + f0, D + f0 + 128).astype(np.float16)
    wpos_h = wslice(Wpos, 0, DH).astype(np.float16)
    # wo: local-head ROWS f0:f0+128 on partitions, cols [ct, 128]
    wo_h = np.empty((DEPTH, 128, KT, 128), np.float16)
    for l in range(DEPTH):
        for ct in range(KT):
            wo_h[l, :, ct, :] = Wo[l, f0:f0 + 128, ct * 128:(ct + 1) * 128]
    w1_h = wslice(W1, 0, 2048).astype(np.float16)
    w2_h = np.empty((DEPTH, 128, 16, 512), np.float16)
    for l in range(DEPTH):
        for hc in range(16):
            w2_h[l, :, hc, :] = W2[l, hc * 128:(hc + 1) * 128, :]
    whead_h = np.zeros((128, KT, VPAD), np.float16)
    for kt in range(KT):
        whead_h[:, kt, :V // 4] = Whead[kt * 128:(kt + 1) * 128, v0:v0 + V // 4]
    wexp_h = np.empty((128, DEPTH, KT), np.float32)
    for l in range(DEPTH):
        for kt in range(KT):
            wexp_h[:, l, kt] = Wexp[l, kt * 128:(kt + 1) * 128, 0]

    def bcol(bvec):  # [DEPTH, c] slice -> [128, DEPTH]
        return np.ascontiguousarray(bvec.T)

    bq_h = bcol(bq[:, f0:f0 + 128] * 0.125)
    bk_h = bcol(bkv[:, f0:f0 + 128])
    bv_h = bcol(bkv[:, D + f0:D + f0 + 128])
    bpos_h = bcol(bpos)                                   # [64, DEPTH]
    bexp_h = np.tile(bexp.reshape(DEPTH, 1), (1, 128)).T.astype(np.float32)
    boq_h = np.empty((128, DEPTH * KT), np.float32)
    b2_h = np.empty((128, DEPTH * KT), np.float32)
    for l in range(DEPTH):
        for ct in range(KT):
            boq_h[:, l * KT + ct] = bo[l, ct * 128:(ct + 1) * 128] * 0.25
            b2_h[:, l * KT + ct] = b2[l, ct * 128:(ct + 1) * 128]
    b1_h = np.empty((128, DEPTH * 16), np.float32)
    for l in range(DEPTH):
        for hc in range(16):
            b1_h[:, l * 16 + hc] = b1[l, hc * 128:(hc + 1) * 128]
    lng_h = np.ascontiguousarray(ln_g.reshape(KT, 128).T)
    lnb_h = np.ascontiguousarray(ln_b.reshape(KT, 128).T)

    return {
        "tok_emb": np.ascontiguousarray(tok_emb),
        "tok_idx": _wrap_idx(tokens),
        "memT": memT, "tpre": tpre,
        "wq": wq_h, "wk": wk_h, "wv": wv_h, "wpos": wpos_h,
        "wexp": wexp_h, "wo": wo_h, "w1": w1_h, "w2": w2_h,
        "whead": whead_h,
        "bq": bq_h, "bk": bk_h, "bv": bv_h, "bpos": bpos_h, "bexp": bexp_h,
        "boq": boq_h, "b1": b1_h, "b2": b2_h,
        "lng": lng_h, "lnb": lnb_h,
    }


RAMP_ = 128.0


def run(inputs, depth=DEPTH, debug=False, trace=False):
    key = (depth, debug)
    if key not in _CACHE:
        _CACHE[key] = build_program(depth=depth, debug=debug)
    nc = _CACHE[key]
    in_maps = [prepare_inputs(c, inputs) for c in range(NC_)]
    res = run_bass_kernel_spmd(nc, in_maps, core_ids=list(range(NC_)),
                               trace=trace)
    logits = np.empty((2, N, V), np.float32)
    for c in range(NC_):
        b, s = c // 4, c % 4
        logits[b, :, (V // 4) * s:(V // 4) * (s + 1)] = \
            res.results[c]["logits_part"][:, :V // 4]
    aux = np.array([res.results[0]["aux_part"][0, 0],
                    res.results[4]["aux_part"][0, 0]], np.float32)
    return logits, aux, res


def kernel(**inputs):
    logits, aux, _ = run(inputs)
    return logits, aux


# revision 28
# speedup vs baseline: 1.1123x; 1.1123x over previous
"""ExpireSpanGPT Bass/Tile kernel for 8 Trainium2 NeuronCores.

Sharding: 2 groups x 4 cores; group g owns batch b=g. Within a group
(TP-4): each core owns 2 heads. Per layer: local-head attention ->
Wo-projection partials (+0.25*x residual) -> ReduceScatter(add) hands
each core its 256-token slice -> MLP on the slice -> AllGather restores
full x^T. Head phase: each core computes logits for its batch over a
8000-wide vocab slice (padded to 8192).

Activations live transposed in SBUF: x^T [128(d-part), kt, tok].
TXL relative-position skew via strided DRAM round trip: Qp rows written
at stride 1152, read back at stride 1151; sentinel columns (-60000)
double as the causal mask.
"""

import numpy as np

# --- NTFF profiling shim: register antenv.axon_hooks if absent -------------
import contextlib as _ctxlib
import ctypes as _ctypes
import os as _os
import sys as _sys
import types as _types


def _make_ntff_hook():
    so = _os.environ.get("AXON_PJRT_SO", "/opt/axon/libaxon_pjrt.so")
    if not _os.path.exists(so):
        return None
    try:
        lib = _ctypes.CDLL(so)
    except OSError:
        return None
    if not hasattr(lib, "axon_start_nrt_profile"):
        return None
    lib.axon_start_nrt_profile.argtypes = [
        _ctypes.POINTER(_ctypes.c_int64), _ctypes.c_size_t]
    lib.axon_start_nrt_profile.restype = _ctypes.c_int64
    lib.axon_stop_nrt_profile.argtypes = [_ctypes.c_char_p]
    lib.axon_stop_nrt_profile.restype = _ctypes.c_int64

    @_ctxlib.contextmanager
    def _hook(output_dir, device_ids):
        import jax
        jax.devices()
        if device_ids:
            ids = (_ctypes.c_int64 * len(device_ids))(*device_ids)
            rc = lib.axon_start_nrt_profile(ids, len(device_ids))
        else:
            rc = lib.axon_start_nrt_profile(None, 0)
        if rc != 0:
            raise RuntimeError(f"axon_start_nrt_profile rc={rc}")
        try:
            yield
        finally:
            n = lib.axon_stop_nrt_profile(output_dir.encode())
            if n < 0:
                raise RuntimeError(f"axon_stop_nrt_profile rc={n}")

    return _hook


try:
    from antenv.axon_hooks import get_axon_ntff_profile_hook  # noqa: F401
except ImportError:
    _m = _types.ModuleType("antenv.axon_hooks")
    _NTFF_HOOK = [None, False]

    def set_axon_ntff_profile_hook(hook):
        _NTFF_HOOK[0] = hook
        _NTFF_HOOK[1] = True

    def get_axon_ntff_profile_hook():
        if not _NTFF_HOOK[1]:
            _NTFF_HOOK[0] = _make_ntff_hook()
            _NTFF_HOOK[1] = True
        return _NTFF_HOOK[0]

    _m.set_axon_ntff_profile_hook = set_axon_ntff_profile_hook
    _m.get_axon_ntff_profile_hook = get_axon_ntff_profile_hook
    _sys.modules["antenv.axon_hooks"] = _m
    try:
        import antenv as _antenv
        _antenv.axon_hooks = _m
    except ImportError:
        pass
# ---------------------------------------------------------------------------

import concourse.bass as bass
import concourse.tile as tile
from concourse import bacc, mybir
from concourse.bass_utils import run_bass_kernel_spmd

FP = mybir.dt.float32
FPR = mybir.dt.float32r
F16 = mybir.dt.float16
I16 = mybir.dt.int16

DEPTH, D, H, DH = 4, 512, 8, 64
N, M, V = 1024, 1024, 32000
KT = D // 128
NC_ = 8
GROUPS = [[0, 1, 2, 3], [4, 5, 6, 7]]
VPAD = 8192
TOKSL = N // 4
QSTRIDE = 1152
SENT = -60000.0
AF = mybir.ActivationFunctionType
OP = mybir.AluOpType

_CACHE = {}


def _sin_emb(n, dim):
    inv = 1.0 / (10000.0 ** (np.arange(0, dim, 2, dtype=np.float64) / dim))
    t = np.arange(n - 1, -1, -1, dtype=np.float64)
    s = t[:, None] * inv[None, :]
    return np.concatenate([np.sin(s), np.cos(s)], axis=-1).astype(np.float32)


def build_program(depth=DEPTH, debug=False):
    nc = bacc.Bacc("TRN2", target_bir_lowering=False, debug=False,
                   num_devices=NC_)

    def din(name, shape, dt=FP):
        return nc.dram_tensor(name, list(shape), dt, kind="ExternalInput")

    def dout(name, shape, dt=FP):
        return nc.dram_tensor(name, list(shape), dt, kind="ExternalOutput")

    tok_emb = din("tok_emb", [V, D])
    tok_idx = din("tok_idx", [128, 64], I16)
    memT = din("memT", [DEPTH, 128, KT, M])
    tpre = din("tpre", [128, DEPTH * 8])
    wq = din("wq", [DEPTH, 128, KT, 128], F16)
    wk = din("wk", [DEPTH, 128, KT, 128], F16)
    wv = din("wv", [DEPTH, 128, KT, 128], F16)
    wpos = din("wpos", [DEPTH, 128, KT, 64], F16)
    wexp = din("wexp", [128, DEPTH, KT])
    wo = din("wo", [DEPTH, 128, KT, 128], F16)   # local-head rows
    w1 = din("w1", [DEPTH, 128, KT, 2048], F16)
    w2 = din("w2", [DEPTH, 128, 16, 512], F16)
    whead = din("whead", [128, KT, VPAD], F16)
    bq = din("bq", [128, DEPTH])
    bk = din("bk", [128, DEPTH])
    bv = din("bv", [128, DEPTH])
    bpos = din("bpos", [64, DEPTH])
    bexp = din("bexp", [128, DEPTH])
    boq = din("boq", [128, DEPTH * KT])      # bo/4: [p, l*4+ct]
    b1 = din("b1", [128, DEPTH * 16])
    b2 = din("b2", [128, DEPTH * KT])
    lng = din("lng", [128, KT])
    lnb = din("lnb", [128, KT])

    logits_part = dout("logits_part", [N, VPAD])
    aux_part = dout("aux_part", [1, 1])
    dbg = {}
    if debug:
        for l in range(depth + 1):
            dbg[l] = dout(f"dbg_x{l}", [128, KT, N])
        dbg["em"] = dout("dbg_em", [128, DEPTH * 8])
        dbg["ao"] = dout("dbg_ao", [128, N])
        dbg["yb"] = dout("dbg_yb", [128, KT, N])
        dbg["mi"] = dout("dbg_mi", [128, KT, TOKSL])

    peT_np = np.zeros((128, KT, N), np.float32)
    pe = _sin_emb(N, D)
    for kt in range(KT):
        peT_np[:, kt, :] = pe[:, kt * 128:(kt + 1) * 128].T
    peT_d = nc.inline_tensor(peT_np.astype(np.float16), "peT_c")
    ident_d = nc.inline_tensor(np.eye(128, dtype=np.float32), "ident_c")
    ones1_d = nc.inline_tensor(np.ones((128, 1), np.float32), "ones1_c")
    onesr_d = nc.inline_tensor(np.ones((1, 128), np.float32), "onesr_c")

    from contextlib import ExitStack
    with tile.TileContext(nc) as tc, ExitStack() as ctx:
        cpool = ctx.enter_context(tc.tile_pool(name="const", bufs=1))
        spool = ctx.enter_context(tc.tile_pool(name="state", bufs=1))
        lpool = ctx.enter_context(tc.tile_pool(name="layer", bufs=1))
        mpool = ctx.enter_context(tc.tile_pool(name="mem", bufs=1))
        apool = ctx.enter_context(tc.tile_pool(name="attn", bufs=2))
        epool = ctx.enter_context(tc.tile_pool(name="attnE", bufs=2))
        wpool = ctx.enter_context(tc.tile_pool(name="wstream", bufs=3))
        psA = ctx.enter_context(tc.tile_pool(name="psA", bufs=3, space="PSUM"))
        psB = ctx.enter_context(tc.tile_pool(name="psB", bufs=2, space="PSUM"))
        psC = ctx.enter_context(tc.tile_pool(name="psC", bufs=1, space="PSUM"))
        psD = ctx.enter_context(tc.tile_pool(name="psD", bufs=2, space="PSUM"))
        dram = ctx.enter_context(tc.tile_pool(name="dram", bufs=1,
                                              space="DRAM"))
        qdram = ctx.enter_context(tc.tile_pool(name="qdram", bufs=16,
                                               space="DRAM"))
        hpool = ctx.enter_context(tc.tile_pool(name="whd", bufs=2))

        def mm512():
            return psA.tile([128, 512], FP, tag="mm512", name="mm512")

        def mmsmall():
            return psB.tile([128, 128], FP, tag="small", name="small")

        peT = cpool.tile([128, KT, N], F16, tag="peT")
        nc.sync.dma_start(peT[:], peT_d[:])
        ident = cpool.tile([128, 128], FP, tag="ident")
        nc.sync.dma_start(ident[:], ident_d[:])
        ones1 = cpool.tile([128, 1], FP, tag="ones1")
        nc.sync.dma_start(ones1[:], ones1_d[:])
        onesr = cpool.tile([1, 128], FP, tag="onesr")
        nc.sync.dma_start(onesr[:], onesr_d[:])
        sent = cpool.tile([128, 128], F16, tag="sent")
        nc.vector.memset(sent[:], SENT)

        def cload(name, src, shape, dt=FP):
            t = cpool.tile(list(shape), dt, tag=name, name=name)
            nc.sync.dma_start(t[:], src[:])
            return t

        tpre_s = cload("tpre", tpre, [128, DEPTH * 8])
        bq_s = cload("bq", bq, [128, DEPTH])
        bk_s = cload("bk", bk, [128, DEPTH])
        bv_s = cload("bv", bv, [128, DEPTH])
        bpos_s = cload("bpos", bpos, [64, DEPTH])
        bexp_s = cload("bexp", bexp, [128, DEPTH])
        boq_s = cload("boq", boq, [128, DEPTH * KT])
        b1_s = cload("b1", b1, [128, DEPTH * 16])
        b2_s = cload("b2", b2, [128, DEPTH * KT])
        lng_s = cload("lng", lng, [128, KT])
        lnb_s = cload("lnb", lnb, [128, KT])
        wexp_s = cload("wexp", wexp, [128, DEPTH, KT])
        idx_s = cpool.tile([128, 64], I16, tag="idx")
        nc.sync.dma_start(idx_s[:], tok_idx[:])

        qpfs = []
        for qi in range(16):
            qf = qdram.tile([128 * QSTRIDE + 1024], F16, tag="qpf")
            qv = qf[:128 * QSTRIDE].rearrange("(r s) -> r s", s=QSTRIDE)
            nc.sync.dma_start(qv[:, 1024:1152], sent[:])
            qpfs.append(qf)

        # ---- embedding gather + transpose into xT ----
        xT = spool.tile([128, KT, N], FP, tag="xT")
        x0 = spool.tile([128, 8, D], FP, tag="ybuf")
        nc.gpsimd.dma_gather(
            out_ap=x0[:], in_ap=tok_emb[:], idxs_ap=idx_s[:],
            num_idxs=N, num_idxs_reg=N, elem_size=D)
        for g in range(8):
            for kt in range(KT):
                ps = mmsmall()
                nc.tensor.transpose(ps[:], x0[:, g, kt * 128:(kt + 1) * 128],
                                    ident[:])
                nc.vector.tensor_copy(
                    out=xT[:, kt, g * 128:(g + 1) * 128], in_=ps[:])
        if debug:
            nc.sync.dma_start(dbg[0][:], xT[:])

        aux_acc = spool.tile([128, 1], FP, tag="aux")
        nc.vector.memset(aux_acc[:], 0.0)
        ybuf = spool.tile([128, KT, N], FP, tag="ybuf")

        for l in range(depth):
            memT_l = mpool.tile([128, KT, M], FP, tag="memT")
            nc.sync.dma_start(memT_l[:], memT[l])
            wq_l = lpool.tile([128, KT, 128], F16, tag="wq")
            nc.sync.dma_start(wq_l[:], wq[l])
            wk_l = lpool.tile([128, KT, 128], F16, tag="wk")
            nc.sync.dma_start(wk_l[:], wk[l])
            wv_l = lpool.tile([128, KT, 128], F16, tag="wv")
            nc.sync.dma_start(wv_l[:], wv[l])
            wpos_l = lpool.tile([128, KT, 64], F16, tag="wpos")
            nc.sync.dma_start(wpos_l[:], wpos[l])
            wo_l = lpool.tile([128, KT, 128], F16, tag="wo")
            nc.sync.dma_start(wo_l[:], wo[l])

            # ---- expire-span gating ----
            sig = lpool.tile([128, 8], FP, tag="sig")
            for jt in range(8):
                pse = psB.tile([128, 1], FP, tag="small", name="exps")
                for kt in range(KT):
                    nc.tensor.matmul(
                        pse[:], lhsT=memT_l[:, kt, jt * 128:(jt + 1) * 128],
                        rhs=wexp_s[:, l, kt:kt + 1],
                        start=(kt == 0), stop=(kt == KT - 1))
                nc.scalar.activation(sig[:, jt:jt + 1], pse[:], AF.Sigmoid,
                                     bias=bexp_s[:, l:l + 1])
            em = lpool.tile([128, 8], FP, tag="em")
            nc.vector.scalar_tensor_tensor(
                em[:], sig[:], 8.0, tpre_s[:, l * 8:(l + 1) * 8],
                OP.mult, OP.add)
            nc.vector.tensor_scalar(em[:], em[:], 1.0, 0.0, OP.min, OP.max)
            if debug:
                nc.sync.dma_start(dbg["em"][:, l * 8:(l + 1) * 8], em[:])
            ind = lpool.tile([128, 8], FP, tag="ind")
            nc.vector.tensor_scalar(ind[:], em[:], 0.0, None, OP.is_gt)
            i2 = lpool.tile([128, 8], FP, tag="ind2")
            nc.vector.tensor_scalar(i2[:], em[:], 1.0, None, OP.is_lt)
            nc.vector.tensor_tensor(ind[:], ind[:], i2[:], OP.mult)
            nc.vector.tensor_tensor(ind[:], ind[:], sig[:], OP.mult)
            red = lpool.tile([128, 1], FP, tag="red")
            nc.vector.tensor_reduce(red[:], ind[:], mybir.AxisListType.X,
                                    OP.add)
            nc.vector.scalar_tensor_tensor(
                aux_acc[:], red[:], 1.0 / 128.0, aux_acc[:], OP.mult, OP.add)

            x16 = lpool.tile([128, KT, N], F16, tag="x16")
            nc.vector.tensor_copy(out=x16[:], in_=xT[:])
            mem16 = lpool.tile([128, KT, M], F16, tag="mem16")
            nc.vector.tensor_copy(out=mem16[:], in_=memT_l[:])

            # ---- kT, vT, qT, posT ----
            kT = lpool.tile([128, 4, 512], F16, tag="kT")
            v_nat = lpool.tile([128, 16, 128], F16, tag="vnat")
            for jc in range(4):
                srct = mem16 if jc < 2 else x16
                off = (jc % 2) * 512
                ps = mm512()
                for kt in range(KT):
                    nc.tensor.matmul(
                        ps[:], lhsT=wk_l[:, kt, :],
                        rhs=srct[:, kt, off:off + 512],
                        start=(kt == 0), stop=(kt == KT - 1))
                nc.vector.tensor_scalar(kT[:, jc, :], ps[:],
                                        bk_s[:, l:l + 1], None, OP.add)
                psv = mm512()
                for kt in range(KT):
                    nc.tensor.matmul(
                        psv[:], lhsT=wv_l[:, kt, :],
                        rhs=srct[:, kt, off:off + 512],
                        start=(kt == 0), stop=(kt == KT - 1))
                vc = lpool.tile([128, 512], FP, tag="vTc")
                nc.vector.tensor_scalar(vc[:], psv[:],
                                        bv_s[:, l:l + 1], None, OP.add)
                for sub in range(4):
                    jt = jc * 4 + sub
                    pst = mmsmall()
                    nc.tensor.transpose(
                        pst[:], vc[:, sub * 128:(sub + 1) * 128], ident[:])
                    if jt < 8:
                        nc.vector.tensor_scalar(v_nat[:, jt, :], pst[:],
                                                em[:, jt:jt + 1], None,
                                                OP.mult)
                    else:
                        nc.vector.tensor_copy(out=v_nat[:, jt, :], in_=pst[:])
            qT = lpool.tile([128, 2, 512], F16, tag="qT")
            for ic in range(2):
                ps = mm512()
                for kt in range(KT):
                    nc.tensor.matmul(
                        ps[:], lhsT=wq_l[:, kt, :],
                        rhs=x16[:, kt, ic * 512:(ic + 1) * 512],
                        start=(kt == 0), stop=(kt == KT - 1))
                nc.vector.tensor_scalar(qT[:, ic, :], ps[:], bq_s[:, l:l + 1],
                                        None, OP.add)
            posT = lpool.tile([128, 2, 512], F16, tag="posT")
            for ic in range(2):
                ps = mm512()
                for kt in range(KT):
                    nc.tensor.matmul(
                        ps[:64, :], lhsT=wpos_l[:, kt, :],
                        rhs=peT[:, kt, ic * 512:(ic + 1) * 512],
                        start=(kt == 0), stop=(kt == KT - 1))
                nc.vector.tensor_scalar(posT[:64, ic, :], ps[:64, :],
                                        bpos_s[:, l:l + 1], None, OP.add)
                nc.sync.dma_start(posT[64:, ic, :], posT[:64, ic, :])

            # ---- attention (2 local heads) ----
            aoT_loc = lpool.tile([128, N], F16, tag="aoT_loc")
            for hh in range(2):
                qr = 64 * hh
                # phase A: all skew round-trips in flight up front
                pds = []
                for it in range(8):
                    i0 = it * 128
                    qsl = qT[qr:qr + 64, it // 4,
                             (it % 4) * 128:(it % 4 + 1) * 128]
                    lo_cc = (1024 - 128 * (it + 1)) // 512
                    qpf = qpfs[hh * 8 + it]
                    qv = qpf[:128 * QSTRIDE].rearrange("(r s) -> r s",
                                                       s=QSTRIDE)
                    qp16 = epool.tile([128, 1024], F16, tag="qp16")
                    for cc in range(lo_cc, 2):
                        ps = mm512()
                        nc.tensor.matmul(ps[:], lhsT=qsl,
                                         rhs=posT[qr:qr + 64, cc, :],
                                         start=True, stop=True)
                        nc.vector.tensor_copy(
                            out=qp16[:, cc * 512:(cc + 1) * 512], in_=ps[:])
                        nc.scalar.dma_start(
                            qv[:, cc * 512:(cc + 1) * 512],
                            qp16[:, cc * 512:(cc + 1) * 512])
                    pw = 128 * (it + 1)
                    pd = apool.tile([128, pw], F16, tag=f"pd{it}",
                                    name=f"pd{it}")
                    base = 1023 - i0
                    shear = qpf[base: base + 128 * (QSTRIDE - 1)
                                ].rearrange("(r s) -> r s", s=QSTRIDE - 1)
                    nc.scalar.dma_start(pd[:, :pw], shear[:, :pw])
                    pds.append(pd)
                # phase B/C per i-tile
                for it in range(8):
                    i0 = it * 128
                    qsl = qT[qr:qr + 64, it // 4,
                             (it % 4) * 128:(it % 4 + 1) * 128]
                    width = 1024 + 128 * (it + 1)
                    jtiles = width // 128
                    njc = (width + 511) // 512
                    pw = 128 * (it + 1)
                    pd = pds[it]

                    E = epool.tile([128, 2048], F16, tag="E")
                    dens = apool.tile([128, 4], FP, tag="dens")
                    for jc in range(njc):
                        w = min(512, width - jc * 512)
                        ps = mm512()
                        nc.tensor.matmul(
                            ps[:, :w], lhsT=qsl,
                            rhs=kT[qr:qr + 64, jc, :w],
                            start=True, stop=True)
                        jj0 = max(0, jc * 512 - 1024)
                        jj1 = min(pw, (jc + 1) * 512 - 1024)
                        if jj1 > jj0:
                            c0 = 1024 + jj0 - jc * 512
                            nc.vector.tensor_tensor(
                                ps[:, c0:c0 + (jj1 - jj0)],
                                ps[:, c0:c0 + (jj1 - jj0)],
                                pd[:, jj0:jj1], OP.add)
                        nc.scalar.activation(
                            E[:, jc * 512:jc * 512 + w], ps[:, :w], AF.Exp,
                            accum_out=dens[:, jc:jc + 1])
                    ET = epool.tile([128, 16, 128], F16, tag="ET")
                    nc.scalar.dma_start_transpose(ET[:, :jtiles, :],
                                                E[:, :width])
                    den1 = apool.tile([128, 1], FP, tag="den1")
                    nc.vector.tensor_reduce(den1[:], dens[:, :njc],
                                            mybir.AxisListType.X, OP.add)
                    rinv = apool.tile([128, 1], FP, tag="rinv")
                    nc.vector.reciprocal(rinv[:], den1[:])
                    po = psD.tile([128, 64], FP, tag="po", name="po")
                    for jt in range(jtiles):
                        nc.tensor.matmul(
                            po[:, :64], lhsT=ET[:, jt, :],
                            rhs=v_nat[:, jt, qr:qr + 64],
                            start=(jt == 0), stop=(jt == jtiles - 1))
                    att = apool.tile([128, 64], FP, tag="att")
                    nc.vector.tensor_scalar(att[:], po[:, :64], rinv[:],
                                            None, OP.mult)
                    pt = mmsmall()
                    nc.tensor.transpose(pt[:64, :], att[:], ident[:])
                    nc.vector.tensor_copy(
                        out=aoT_loc[qr:qr + 64, i0:i0 + 128], in_=pt[:64, :])
            if debug and l == 0:
                nc.sync.dma_start(dbg["ao"][:], aoT_loc[:])

            # ---- Wo projection partials + 0.25*x + bo/4 ----
            for ct in range(KT):
                for ic in range(2):
                    ps = mm512()
                    nc.tensor.matmul(
                        ps[:], lhsT=wo_l[:, ct, :],
                        rhs=aoT_loc[:, ic * 512:(ic + 1) * 512],
                        start=True, stop=True)
                    sl = slice(ic * 512, (ic + 1) * 512)
                    nc.vector.scalar_tensor_tensor(
                        ybuf[:, ct, sl], xT[:, ct, sl], 0.25, ps[:],
                        OP.mult, OP.add)
                nc.vector.tensor_scalar(
                    ybuf[:, ct, :], ybuf[:, ct, :],
                    boq_s[:, l * KT + ct:l * KT + ct + 1], None, OP.add)

            if debug and l == 0:
                nc.sync.dma_start(dbg["yb"][:], ybuf[:])
            # ---- ReduceScatter -> my 256-token slice ----
            rs_in = dram.tile([4, 128, KT, TOKSL], FP, tag="rsi")
            rs_out = dram.tile([128, KT, TOKSL], FP, tag="rso")
            for sr in range(4):
                nc.gpsimd.dma_start(
                    rs_in[sr], ybuf[:, :, sr * TOKSL:(sr + 1) * TOKSL])
            nc.gpsimd.collective_compute(
                "ReduceScatter", OP.add, replica_groups=GROUPS,
                ins=[rs_in.opt()], outs=[rs_out.opt()])
            mlp_in = lpool.tile([128, KT, TOKSL], FP, tag="mlpin")
            nc.sync.dma_start(mlp_in[:], rs_out[:])
            if debug and l == 0:
                nc.sync.dma_start(dbg["mi"][:], mlp_in[:])

            mlp16 = lpool.tile([128, KT, TOKSL], F16, tag="mlp16")
            nc.vector.tensor_copy(out=mlp16[:], in_=mlp_in[:])

            # ---- MLP on slice ----
            h1full = lpool.tile([128, 16, TOKSL], F16, tag="h1")
            for hc in range(16):
                w1t = wpool.tile([128, KT, 128], F16, tag="w1t")
                nc.sync.dma_start(w1t[:],
                                  w1[l, :, :, hc * 128:(hc + 1) * 128])
                ps = mm512()
                for kt in range(KT):
                    nc.tensor.matmul(
                        ps[:, :TOKSL], lhsT=w1t[:, kt, :],
                        rhs=mlp16[:, kt, :],
                        start=(kt == 0), stop=(kt == KT - 1))
                nc.scalar.activation(
                    h1full[:, hc, :], ps[:, :TOKSL], AF.Gelu,
                    bias=b1_s[:, l * 16 + hc:l * 16 + hc + 1])
            xout = lpool.tile([128, KT, TOKSL], FP, tag="xout")
            for ct in range(KT):
                w2t = hpool.tile([128, 16, 128], F16, tag="w2t")
                nc.sync.dma_start(w2t[:], w2[l, :, :, ct * 128:(ct + 1) * 128])
                zp = psC.tile([128, 256], FP, tag="acc256", name="zp")
                for hc in range(16):
                    nc.tensor.matmul(
                        zp[:], lhsT=w2t[:, hc, :],
                        rhs=h1full[:, hc, :],
                        start=(hc == 0), stop=(hc == 15))
                nc.vector.scalar_tensor_tensor(
                    xout[:, ct, :], zp[:],
                    b2_s[:, l * KT + ct:l * KT + ct + 1],
                    mlp_in[:, ct, :], OP.add, OP.add)

            # ---- AllGather token slices -> full xT ----
            ag_in = dram.tile([128, KT, TOKSL], FP, tag="agi")
            ag_out = dram.tile([4, 128, KT, TOKSL], FP, tag="ago")
            nc.gpsimd.dma_start(ag_in[:], xout[:])
            nc.gpsimd.collective_compute(
                "AllGather", OP.bypass, replica_groups=GROUPS,
                ins=[ag_in.opt()], outs=[ag_out.opt()])
            for sr in range(4):
                nc.sync.dma_start(
                    xT[:, :, sr * TOKSL:(sr + 1) * TOKSL], ag_out[sr])
            if debug:
                nc.sync.dma_start(dbg[l + 1][:], xT[:])

        # ---- final layernorm (feature dim = partitions) ----
        xsq = spool.tile([128, KT, N], FP, tag="ybuf")
        nc.scalar.activation(xsq[:], xT[:], AF.Square)
        mean1 = spool.tile([1, N], FP, tag="mean1")
        scr = spool.tile([1, N], FP, tag="scr")
        rstd1 = spool.tile([1, N], FP, tag="rstd1")
        for ic in range(2):
            sl = slice(ic * 512, (ic + 1) * 512)
            ps = mm512()
            for kt in range(KT):
                nc.tensor.matmul(
                    ps[:1, :], lhsT=ones1[:],
                    rhs=xT[:, kt, sl],
                    start=(kt == 0), stop=(kt == KT - 1))
            nc.vector.tensor_scalar(mean1[:, sl], ps[:1, :], 1.0 / 512.0,
                                    None, OP.mult)
            ps2 = mm512()
            for kt in range(KT):
                nc.tensor.matmul(
                    ps2[:1, :], lhsT=ones1[:],
                    rhs=xsq[:, kt, sl],
                    start=(kt == 0), stop=(kt == KT - 1))
            nc.vector.tensor_scalar(scr[:, sl], ps2[:1, :], 1.0 / 512.0,
                                    None, OP.mult)
        # var = E[x^2] - mean^2 ; rstd = 1/sqrt(var+eps)
        nc.vector.tensor_tensor(rstd1[:], mean1[:], mean1[:], OP.mult)
        nc.vector.tensor_tensor(scr[:], scr[:], rstd1[:], OP.subtract)
        eps1 = spool.tile([1, 1], FP, tag="eps1")
        nc.vector.memset(eps1[:], 1e-5)
        nc.scalar.activation(rstd1[:], scr[:], AF.Sqrt, bias=eps1[:])
        nc.vector.reciprocal(scr[:], rstd1[:])
        meanb = spool.tile([128, N], F16, tag="meanb")
        rstdb = spool.tile([128, N], F16, tag="rstdb")
        for ic in range(2):
            sl = slice(ic * 512, (ic + 1) * 512)
            ps = mm512()
            nc.tensor.matmul(ps[:], lhsT=onesr[:], rhs=mean1[:, sl],
                             start=True, stop=True)
            nc.vector.tensor_copy(out=meanb[:, sl], in_=ps[:])
            ps2 = mm512()
            nc.tensor.matmul(ps2[:], lhsT=onesr[:], rhs=scr[:, sl],
                             start=True, stop=True)
            nc.vector.tensor_copy(out=rstdb[:, sl], in_=ps2[:])
        xn = spool.tile([128, KT, N], F16, tag="xn16")
        for kt in range(KT):
            nc.vector.tensor_tensor(xn[:, kt, :], xT[:, kt, :], meanb[:],
                                    OP.subtract)
            nc.vector.tensor_tensor(xn[:, kt, :], xn[:, kt, :], rstdb[:],
                                    OP.mult)
            nc.vector.tensor_scalar(xn[:, kt, :], xn[:, kt, :],
                                    lng_s[:, kt:kt + 1], lnb_s[:, kt:kt + 1],
                                    OP.mult, OP.add)

        # ---- head: logits_part[tok, vc] ----
        for vc in range(VPAD // 256):
            wht = hpool.tile([128, KT, 256], F16, tag="wht")
            nc.sync.dma_start(wht[:], whead[:, :, vc * 256:(vc + 1) * 256])
            for tt in range(8):
                ps = mm512()
                for kt in range(KT):
                    nc.tensor.matmul(
                        ps[:, :256], lhsT=xn[:, kt, tt * 128:(tt + 1) * 128
                                             ],
                        rhs=wht[:, kt, :],
                        start=(kt == 0), stop=(kt == KT - 1))
                ob = epool.tile([128, 256], FP, tag="ob")
                nc.vector.tensor_copy(out=ob[:], in_=ps[:, :256])
                nc.scalar.dma_start(
                    logits_part[tt * 128:(tt + 1) * 128,
                                vc * 256:(vc + 1) * 256], ob[:])

        # ---- aux scalar (partition-reduce via ones-matmul) ----
        aux_ps = psB.tile([128, 128], FP, tag="small", name="auxps")
        nc.tensor.matmul(aux_ps[:1, :1], lhsT=ones1[:], rhs=aux_acc[:],
                         start=True, stop=True)
        aux1 = spool.tile([1, 1], FP, tag="aux1")
        nc.vector.tensor_copy(out=aux1[:], in_=aux_ps[:1, :1])
        nc.sync.dma_start(aux_part[:], aux1[:])

    nc.compile()
    return nc


def _wrap_idx(idx):
    """[1024] -> [128, 64] int16 wrapped in 16 partitions, replicated x8."""
    w = idx.astype(np.int16).reshape(64, 16).T          # [16, 64]
    return np.ascontiguousarray(np.tile(w, (8, 1)))     # [128, 64]


def _tile_T(x):
    """[rows, D] -> [128, D//128, rows] transposed-tiled."""
    rows, d = x.shape
    out = np.empty((128, d // 128, rows), np.float32)
    for kt in range(d // 128):
        out[:, kt, :] = x[:, kt * 128:(kt + 1) * 128].T
    return out


def prepare_inputs(core, inputs):
    b, s = core // 4, core % 4
    f0 = 128 * s                      # local head-feature offset (2 heads)
    t0 = TOKSL * s                    # token slice (only used via collective)
    v0 = (V // 4) * s                 # vocab slice

    tokens = np.asarray(inputs["tokens"])[b]
    mems = np.asarray(inputs["mems"], np.float32)
    times = np.asarray(inputs["times"])
    Wq = np.asarray(inputs["Wq"], np.float32)
    bq = np.asarray(inputs["bq"], np.float32)
    Wkv = np.asarray(inputs["Wkv"], np.float32)
    bkv = np.asarray(inputs["bkv"], np.float32)
    Wo = np.asarray(inputs["Wo"], np.float32)
    bo = np.asarray(inputs["bo"], np.float32)
    Wpos = np.asarray(inputs["Wpos"], np.float32)
    bpos = np.asarray(inputs["bpos"], np.float32)
    Wexp = np.asarray(inputs["Wexp"], np.float32)
    bexp = np.asarray(inputs["bexp"], np.float32)
    W1 = np.asarray(inputs["W1"], np.float32)
    b1 = np.asarray(inputs["b1"], np.float32)
    W2 = np.asarray(inputs["W2"], np.float32)
    b2 = np.asarray(inputs["b2"], np.float32)
    ln_g = np.asarray(inputs["ln_g"], np.float32)
    ln_b = np.asarray(inputs["ln_b"], np.float32)
    Whead = np.asarray(inputs["Whead"], np.float32)
    tok_emb = np.asarray(inputs["tok_emb"], np.float32)

    memT = np.stack([_tile_T(mems[l, b]) for l in range(DEPTH)])
    tpre = np.empty((128, DEPTH * 8), np.float32)
    for l in range(DEPTH):
        tl = times[l, b].astype(np.float32)              # [1024]
        tpre[:, l * 8:(l + 1) * 8] = 1.0 - tl.reshape(8, 128).T / RAMP_

    def wslice(W, c0, c1, scale=1.0):
        # W [DEPTH, 512, c] -> [DEPTH, 128, KT, c1-c0]
        out = np.empty((DEPTH, 128, KT, c1 - c0), np.float32)
        for l in range(DEPTH):
            for kt in range(KT):
                out[l, :, kt, :] = W[l, kt * 128:(kt + 1) * 128, c0:c1] * scale
        return np.ascontiguousarray(out)

    wq_h = wslice(Wq, f0, f0 + 128, 0.125).astype(np.float16)
    wk_h = wslice(Wkv, f0, f0 + 128).astype(np.float16)
    wv_h = wslice(Wkv, D + f0, D + f0 + 128).astype(np.float16)
    wpos_h = wslice(Wpos, 0, DH).astype(np.float16)
    # wo: local-head ROWS f0:f0+128 on partitions, cols [ct, 128]
    wo_h = np.empty((DEPTH, 128, KT, 128), np.float16)
    for l in range(DEPTH):
        for ct in range(KT):
            wo_h[l, :, ct, :] = Wo[l, f0:f0 + 128, ct * 128:(ct + 1) * 128]
    w1_h = wslice(W1, 0, 2048).astype(np.float16)
    w2_h = np.empty((DEPTH, 128, 16, 512), np.float16)
    for l in range(DEPTH):
        for hc in range(16):
            w2_h[l, :, hc, :] = W2[l, hc * 128:(hc + 1) * 128, :]
    whead_h = np.zeros((128, KT, VPAD), np.float16)
    for kt in range(KT):
        whead_h[:, kt, :V // 4] = Whead[kt * 128:(kt + 1) * 128, v0:v0 + V // 4]
    wexp_h = np.empty((128, DEPTH, KT), np.float32)
    for l in range(DEPTH):
        for kt in range(KT):
            wexp_h[:, l, kt] = Wexp[l, kt * 128:(kt + 1) * 128, 0]

    def bcol(bvec):  # [DEPTH, c] slice -> [128, DEPTH]
        return np.ascontiguousarray(bvec.T)

    bq_h = bcol(bq[:, f0:f0 + 128] * 0.125)
    bk_h = bcol(bkv[:, f0:f0 + 128])
    bv_h = bcol(bkv[:, D + f0:D + f0 + 128])
    bpos_h = bcol(bpos)                                   # [64, DEPTH]
    bexp_h = np.tile(bexp.reshape(DEPTH, 1), (1, 128)).T.astype(np.float32)
    boq_h = np.empty((128, DEPTH * KT), np.float32)
    b2_h = np.empty((128, DEPTH * KT), np.float32)
    for l in range(DEPTH):
        for ct in range(KT):
            boq_h[:, l * KT + ct] = bo[l, ct * 128:(ct + 1) * 128] * 0.25
            b2_h[:, l * KT + ct] = b2[l, ct * 128:(ct + 1) * 128]
    b1_h = np.empty((128, DEPTH * 16), np.float32)
    for l in range(DEPTH):
        for hc in range(16):
            b1_h[:, l * 16 + hc] = b1[l, hc * 128:(hc + 1) * 128]
    lng_h = np.ascontiguousarray(ln_g.reshape(KT, 128).T)
    lnb_h = np.ascontiguousarray(ln_b.reshape(KT, 128).T)

    return {
        "tok_emb": np.ascontiguousarray(tok_emb),
        "tok_idx": _wrap_idx(tokens),
        "memT": memT, "tpre": tpre,
        "wq": wq_h, "wk": wk_h, "wv": wv_h, "wpos": wpos_h,
        "wexp": wexp_h, "wo": wo_h, "w1": w1_h, "w2": w2_h,
        "whead": whead_h,
        "bq": bq_h, "bk": bk_h, "bv": bv_h, "bpos": bpos_h, "bexp": bexp_h,
        "boq": boq_h, "b1": b1_h, "b2": b2_h,
        "lng": lng_h, "lnb": lnb_h,
    }


RAMP_ = 128.0


def run(inputs, depth=DEPTH, debug=False, trace=False):
    key = (depth, debug)
    if key not in _CACHE:
        _CACHE[key] = build_program(depth=depth, debug=debug)
    nc = _CACHE[key]
    in_maps = [prepare_inputs(c, inputs) for c in range(NC_)]
    res = run_bass_kernel_spmd(nc, in_maps, core_ids=list(range(NC_)),
                               trace=trace)
    logits = np.empty((2, N, V), np.float32)
    for c in range(NC_):
        b, s = c // 4, c % 4
        logits[b, :, (V // 4) * s:(V // 4) * (s + 1)] = \
            res.results[c]["logits_part"][:, :V // 4]
    aux = np.array([res.results[0]["aux_part"][0, 0],
                    res.results[4]["aux_part"][0, 0]], np.float32)
    return logits, aux, res


def kernel(**inputs):
    logits, aux, _ = run(inputs)
    return logits, aux
